# revision 6
# baseline (speedup 1.0000x reference)
"""Trainium2 Bass kernel: 8 independent 3x3 filters applied to every channel.

Reference op: x[B=8, C=32, 224, 224], W[1, 8, 3, 3], Bv[8]
  -> y[B, 8*C, 222, 222],  y[b, d*C+c, i, j] = sum_{u,v} x[b,c,i+u,j+v] W[0,d,u,v] + Bv[d]

Sharding: data-parallel over batch B across the 8 cores (core k takes x[k]).

Per-core formulation (v-folded contraction):
  The kernel-width taps v are folded into the matmul contraction dim by
  holding three column-shifted copies of a 34-row block of the image in
  SBUF partitions: partition (v, r') holds x[c, r0+r', j+v].  A constant
  ones partition (102) carries the bias.  K = 3*34 + 1 = 103.
  M = 128 = 8 filters x 16 row-groups via the banded weight matrix
    LW[(v, r'), tt, (d, rl)] = W[d, r'-(2*rl+tt), v]   (band of 3 in u)
  so ONE matmul per (block, pair, tt) produces 128 output rows-groups x
  444 columns (2 channels x 222) with all 9 taps contracted: TensorE
  streams each output column once instead of 3x.

  7 row-blocks of 32 output rows cover 224 rows (2 garbage rows dropped
  on host).  Inputs are host-permuted to [rows, C, cols] so each
  partition's DMA run is 32*224*2B = 14336B; the three shifted copies are
  element-offset views of the same padded buffer.  Outputs accumulate in
  a per-pair SBUF tile [128, 14, 444] and leave as one DMA per pair with
  12432B per-partition runs (bf16).  PSUM->SBUF copies (pure cast, bias
  already in the matmul) alternate between DVE and Act in 4-bank chunks.
"""

import os
import numpy as np

B, C, H, W_IN = 8, 32, 224, 224
ND, KS = 8, 3
HO, WO = 222, 222
NCORES = 8

NRL = 16            # row-groups per block
NT = 2              # rows per row-group
NB = 7              # row blocks (7*32 = 224 >= 222 output rows)
BR = NRL * NT       # output rows per block (32)
RSPAN = BR + KS - 1 # input rows per block copy (34)
KP = 3 * RSPAN + 1  # matmul contraction partitions (103, incl. ones row)
MM = ND * NRL       # matmul M (128)
NPAIR = C // 2      # image pairs per core (16)
NG = NB * NT        # psum tiles per pair (14), g = b*NT + tt
NW = 2 * WO         # matmul N (444)
PAD_ROWS = 228      # padded input rows

_PROG_CACHE = {}


def _build(out_mode: str, qscale: float):
    """Build+compile the per-core Bass program.

    out_mode: 'bf16' (output cast) or 'i8' (output quantized by qscale,
    host divides it back out).
    """
    import concourse.mybir as mybir
    import concourse.tile as tile
    from concourse import bacc

    dt = mybir.dt
    io_dt = dt.bfloat16
    out_dt = dt.bfloat16 if out_mode == "bf16" else dt.int8

    nc = bacc.Bacc("TRN2", target_bir_lowering=False, debug=False)
    xin = nc.dram_tensor("xin", [NB, KP, C, W_IN], io_dt,
                         kind="ExternalInput")
    lw = nc.dram_tensor("lw", [KP, NT, MM], io_dt, kind="ExternalInput")
    yout = nc.dram_tensor("yout", [NPAIR, MM, NG, NW], out_dt,
                          kind="ExternalOutput")

    with tile.TileContext(nc) as tc:
        with (
            tc.tile_pool(name="const", bufs=1) as constp,
            tc.tile_pool(name="inp", bufs=1) as inp,
            tc.tile_pool(name="outp", bufs=3) as outp,
            tc.tile_pool(name="psum", bufs=2, space="PSUM") as psp,
        ):
            lwt = constp.tile([KP, NT, MM], io_dt, name="lwt")
            nc.sync.dma_start(lwt[:], lw[:])

            # 7 persistent block tiles: partition (v, r') = row r0+r'
            # shifted v cols; partition 102 = ones (bias input).  All
            # loads on the two HWDGE rings (SWDGE descriptor gen is far
            # too slow for bulk traffic).
            tiles = []
            for b in range(NB):
                t = inp.tile([KP, C, W_IN], io_dt, name=f"t{b}", tag=f"t{b}")
                # >=4KB M2S descriptors pin to 1-2 DMA engines; splitting
                # the 14336B partition runs into 3584B descriptors spreads
                # the load across the engine pool (max_dma_last_dim is
                # ignored for symbolic tile APs, so split manually)
                for q in range(4):
                    eng = nc.sync if (4 * b + q) % 2 == 0 else nc.scalar
                    eng.dma_start(t[:, 8 * q:8 * q + 8, :],
                                  xin[b, :, 8 * q:8 * q + 8, :])
                tiles.append(t)

            def drain(eng, dst, src):
                # PSUM->SBUF cast (bias folded into the matmul); int8
                # mode also applies the quantization scale.
                if out_mode == "bf16":
                    if eng is nc.vector:
                        eng.tensor_copy(dst, src)
                    else:
                        eng.copy(dst, src)
                else:
                    if eng is nc.vector:
                        eng.tensor_scalar_mul(dst, src, qscale)
                    else:
                        eng.mul(dst, src, qscale)

            for pr in range(NPAIR):
                acc = outp.tile([MM, NG, NW], out_dt, name="acc", tag="acc")
                for c in range(4):      # 4-bank psum chunks over g
                    g0, g1 = 4 * c, min(4 * c + 4, NG)
                    ps = psp.tile([MM, 4, 512], dt.float32, name="ps",
                                  tag="ps")
                    for i, g in enumerate(range(g0, g1)):
                        b, tt = g // NT, g % NT
                        nc.tensor.matmul(
                            ps[:, i, 0:NW],
                            lwt[:, tt, :],
                            tiles[b][:, 2 * pr:2 * pr + 2, 0:WO],
                            start=True, stop=True,
                        )
                    # split each chunk's drain across DVE and Act so the
                    # psum recycle never paces the matmul stream
                    n = g1 - g0
                    h = n // 2
                    drain(nc.vector, acc[:, g0:g0 + h, :], ps[:, 0:h, 0:NW])
                    drain(nc.scalar, acc[:, g0 + h:g1, :],
                          ps[:, h:n, 0:NW])
                oeng = nc.sync if pr % 2 == 0 else nc.scalar
                oeng.dma_start(yout[pr, :, :, :], acc[:])

    nc.compile()
    return nc


def _get_prog(out_mode: str, qscale: float):
    key = (out_mode, round(float(qscale), 9))
    if key not in _PROG_CACHE:
        _PROG_CACHE[key] = _build(out_mode, qscale)
    return _PROG_CACHE[key]


def _host_weights(W: np.ndarray, Bv: np.ndarray):
    """LW[(v, r'), tt, (d, rl)] = W[0, d, u, v], u = r' - (2*rl + tt);
    ones row KP-1 carries Bv[d]."""
    import ml_dtypes
    W = np.asarray(W, np.float32)
    LW = np.zeros((KP, NT, MM), np.float32)
    for v in range(3):
        for tt in range(NT):
            for d in range(ND):
                for rl in range(NRL):
                    for u in range(3):
                        rp = 2 * rl + tt + u
                        LW[RSPAN * v + rp, tt, NRL * d + rl] = W[0, d, u, v]
    for tt in range(NT):
        for d in range(ND):
            LW[KP - 1, tt, NRL * d:NRL * (d + 1)] = np.float32(Bv[d])
    return np.ascontiguousarray(LW.astype(ml_dtypes.bfloat16))


def _host_x(xk: np.ndarray):
    """Prepack core input [C, H, W] into the replicated shifted block
    layout [NB, KP, C, W] bf16: row (v, r') of block b = x rows permuted
    to [H, C, W], flat-shifted by v elements; row KP-1 = ones (bias)."""
    import ml_dtypes
    xp = np.ascontiguousarray(np.transpose(xk, (1, 0, 2)))  # [H, C, W]
    flat = np.zeros(PAD_ROWS * C * W_IN + 2, dtype=ml_dtypes.bfloat16)
    flat[:H * C * W_IN] = xp.astype(ml_dtypes.bfloat16).ravel()
    rep = np.empty((NB, KP, C * W_IN), dtype=ml_dtypes.bfloat16)
    rw = C * W_IN
    for b in range(NB):
        for v in range(3):
            o = BR * b * rw + v
            rep[b, RSPAN * v:RSPAN * (v + 1), :] = \
                flat[o:o + RSPAN * rw].reshape(RSPAN, rw)
    rep[:, KP - 1, :] = np.float32(1.0)
    return np.ascontiguousarray(rep.reshape(NB, KP, C, W_IN))


def _est_ymax(x: np.ndarray, W: np.ndarray, Bv: np.ndarray) -> float:
    """Cheap strided-subsample conv to bound |y|max for int8 scaling."""
    xs = x[:, :, :, :]
    acc = None
    for u in range(3):
        for v in range(3):
            sl = xs[:, :, u:u + HO:4, v:v + WO:4]
            term = W[0, :, u, v][None, :, None, None, None] * sl[:, None]
            acc = term if acc is None else acc + term
    acc = acc + np.asarray(Bv, np.float32)[None, :, None, None, None]
    return float(np.abs(acc).max())


def kernel(x, W, Bv, mode: str | None = None, _trace: bool = False):
    from concourse.bass_utils import run_bass_kernel_spmd

    out_mode = mode or os.environ.get("DCONV_MODE", "bf16")
    if out_mode not in ("bf16", "i8"):
        out_mode = "bf16"
    x = np.asarray(x, np.float32)
    W = np.asarray(W, np.float32)
    Bv = np.asarray(Bv, np.float32)

    if out_mode == "i8":
        ymax = _est_ymax(x, W, Bv) * 1.35
        qscale = 127.0 / ymax
    else:
        qscale = 1.0

    nc = _get_prog(out_mode, qscale)
    LW = _host_weights(W, Bv)
    in_maps = []
    for k in range(NCORES):
        in_maps.append({"xin": _host_x(x[k]), "lw": LW})
    res = run_bass_kernel_spmd(nc, in_maps, core_ids=list(range(NCORES)),
                               trace=_trace)
    # yout [pair, (d, rl), (b, tt), (im, j)] -> y[d*32+2*pair+im,
    # 32*b+2*rl+tt, j], drop rows 222/223.
    outs = []
    for k in range(NCORES):
        arr = np.asarray(res.results[k]["yout"]).astype(np.float32)
        if out_mode == "i8":
            arr *= (1.0 / qscale)
        arr = arr.reshape(NPAIR, ND, NRL, NB, NT, 2, WO)
        arr = arr.transpose(1, 0, 5, 3, 2, 4, 6).reshape(ND * C, NB * BR, WO)
        outs.append(np.ascontiguousarray(arr[:, :HO, :]))
    y = np.stack(outs, axis=0)
    if _trace:
        return y, res
    return y
